# revision 10
# baseline (speedup 1.0000x reference)
"""Masked multi-head attention (B=4, S=2048, H=16, d_k=64) on 8 TRN2 NeuronCores.

Sharding: core c handles batch b = c//2 and head-group hg = c%2 (8 heads each).
Device algorithm (per core), all layouts chosen so no on-chip transposes are
needed:
  scoresT[k, q] = K @ Q^T        (contraction over d=64; two heads row-packed
                                  into the 128x128 PE array at rows 0-63/64-127)
  E = exp(scoresT / 8) * maskT   (ACT exp straight from PSUM -> bf16 SBUF;
                                  mask multiply on DVE/GPSIMD; no max-subtract:
                                  scores are ~N(0,1), exp is safe in fp32)
  outT[d, q], Z[q] = V_aug^T-style matmul: lhsT = [V | ones] (M=65), rhs = E,
                                  accumulated over 16 k-tiles in PSUM
  out = outT * (1/Z)             (fast reciprocal + DMA partition-broadcast)

Host side only reshapes/transposes/casts (sharding prep): q/k passed
pre-transposed [512, 2048] bf16, v [2048, 512] bf16, mask transposed bf16.
Output returned transposed [512, 2048] fp32 per core and untransposed on host.
"""

import sys

sys.path.insert(0, "/opt/trn_rl_repo")

import numpy as np
import ml_dtypes

import concourse.bass as bass
import concourse.tile as tile
import concourse.mybir as mybir
from concourse import bacc
from concourse import bass_utils

BF16 = mybir.dt.bfloat16
F32 = mybir.dt.float32

# Model dims
S = 2048          # sequence length
DK = 64           # head dim
HPC = 8           # heads per core
N_CORES = 8
QW = 512          # q-tile width (matmul moving free dim / one PSUM bank)
P = 128           # partitions / k-tile height

# Tuning knobs
GPSIMD_EVERY = 3     # every GPSIMD_EVERY'th k-tile's mask-multiply goes to GPSIMD
TRACE = False        # set by test harness to capture an NTFF profile
LAST_RESULTS = None  # BassKernelResults of the most recent run


def build_program(s=S, hpc=HPC):
    """Build the SPMD Bass/Tile program (identical on all cores)."""
    kt_n = s // P         # k-tiles
    qt_n = s // QW        # q-tiles
    pairs = hpc // 2
    hd = hpc * DK         # per-core model cols (512)

    nc = bacc.Bacc("TRN2", debug=False)
    qT = nc.dram_tensor("qT", [hd, s], BF16, kind="ExternalInput").ap()
    kT = nc.dram_tensor("kT", [hd, s], BF16, kind="ExternalInput").ap()
    v = nc.dram_tensor("v", [s, hd], BF16, kind="ExternalInput").ap()
    mT = nc.dram_tensor("mT", [s, s], BF16, kind="ExternalInput").ap()
    outT = nc.dram_tensor("outT", [hd, s], F32, kind="ExternalOutput").ap()

    Exp = mybir.ActivationFunctionType.Exp
    Log = mybir.ActivationFunctionType.Ln

    with tile.TileContext(nc) as tc:
        with (
            tc.tile_pool(name="resident", bufs=1) as resident,
            tc.tile_pool(name="maskp", bufs=2) as maskp,
            tc.tile_pool(name="erawp", bufs=3) as erawp,
            tc.tile_pool(name="ep", bufs=3) as ep,
            tc.tile_pool(name="rcpp", bufs=2) as rcpp,
            tc.tile_pool(name="osbp", bufs=2) as osbp,
            tc.tile_pool(name="psum_s", bufs=2, space="PSUM") as psum_s,
            tc.tile_pool(name="psum_o", bufs=2, space="PSUM") as psum_o,
        ):
            # ---- resident loads ----
            # qT_sb/kT_sb: [128, pairs*s]; pair p at cols [p*s:(p+1)*s].
            # Partitions 0-63 hold head 2p's (d, s) rows, 64-127 head 2p+1's.
            qT_sb = resident.tile([P, pairs * s], BF16)
            kT_sb = resident.tile([P, pairs * s], BF16)
            for p in range(pairs):
                nc.sync.dma_start(qT_sb[:, p * s:(p + 1) * s],
                                  qT[p * P:(p + 1) * P, :])
                nc.sync.dma_start(kT_sb[:, p * s:(p + 1) * s],
                                  kT[p * P:(p + 1) * P, :])
            # v_sb: [128, hpc*kt_n*128]; slot (h, kt) holds [V_tile | ones*64].
            # The 64 ones-columns make the attn@V matmul emit Z (the softmax
            # denominator) replicated across PSUM rows 64-127, so the
            # reciprocal+multiply need no partition broadcast.
            v_sb = resident.tile([P, hpc * kt_n * P], BF16)
            v_sb3 = v_sb.rearrange("p (t e) -> p t e", e=P)
            nc.gpsimd.memset(v_sb3[:, :, 64:128], 1.0)
            v_src = v.rearrange("(kt p) c -> p kt c", p=P)
            for h in range(hpc):
                dst = v_sb[:, h * kt_n * P:(h + 1) * kt_n * P]
                dst3 = dst.rearrange("p (kt e) -> p kt e", e=P)
                nc.sync.dma_start(dst3[:, :, 0:64],
                                  v_src[:, :, h * DK:(h + 1) * DK])

            for qt in range(qt_n):
                # maskT window for this q-tile: [128, kt_n*QW] bf16
                m_sb = maskp.tile([P, kt_n * QW], BF16)
                for kt in range(kt_n):
                    nc.sync.dma_start(
                        m_sb[:, kt * QW:(kt + 1) * QW],
                        mT[kt * P:(kt + 1) * P, qt * QW:(qt + 1) * QW])

                for p in range(pairs):
                    hA, hB = 2 * p, 2 * p + 1
                    # one 2-bank accumulator for the pair: head A cols 0:QW,
                    # head B cols QW:2QW; rows 64-127 accumulate Z (ones-cols)
                    o_ps = psum_o.tile([P, 2 * QW], F32, tag="ops")
                    for kt in range(kt_n):
                        # scoresT for this k-tile, both heads side by side
                        s_ps = psum_s.tile([P, 2 * QW], F32)
                        nc.tensor.matmul(
                            s_ps[:, 0:QW],
                            lhsT=kT_sb[0:64, p * s + kt * P: p * s + (kt + 1) * P],
                            rhs=qT_sb[0:64, p * s + qt * QW: p * s + (qt + 1) * QW],
                            start=True, stop=True)
                        nc.tensor.matmul(
                            s_ps[:, QW:2 * QW],
                            lhsT=kT_sb[64:128, p * s + kt * P: p * s + (kt + 1) * P],
                            rhs=qT_sb[64:128, p * s + qt * QW: p * s + (qt + 1) * QW],
                            start=True, stop=True)
                        # E_raw = exp(scoresT / 8)  (PSUM fp32 -> SBUF bf16)
                        e_raw = erawp.tile([P, 2 * QW], BF16)
                        nc.scalar.activation(e_raw[:], s_ps[:], Exp, scale=0.125)
                        # E = E_raw * maskT  (same mask tile for both heads)
                        e = ep.tile([P, 2 * QW], BF16)
                        msl = m_sb[:, kt * QW:(kt + 1) * QW]
                        eng = nc.gpsimd if kt % GPSIMD_EVERY == GPSIMD_EVERY - 1 \
                            else nc.vector
                        eng.tensor_mul(e[:, 0:QW], e_raw[:, 0:QW], msl)
                        eng.tensor_mul(e[:, QW:2 * QW], e_raw[:, QW:2 * QW], msl)
                        # outT/Z accumulation: [V|1]^T contribution of this k-tile
                        vofsA = (hA * kt_n + kt) * P
                        vofsB = (hB * kt_n + kt) * P
                        nc.tensor.matmul(
                            o_ps[:, 0:QW], lhsT=v_sb[:, vofsA:vofsA + P],
                            rhs=e[:, 0:QW],
                            start=(kt == 0), stop=(kt == kt_n - 1))
                        nc.tensor.matmul(
                            o_ps[:, QW:2 * QW], lhsT=v_sb[:, vofsB:vofsB + P],
                            rhs=e[:, QW:2 * QW],
                            start=(kt == 0), stop=(kt == kt_n - 1))
                    # normalize both heads: 1/Z = exp(-ln Z) on ACT (Log and
                    # Exp share one table set; custom-DVE recip is broken on
                    # HW through this compile path)
                    lnz = rcpp.tile([64, 2 * QW], F32, tag="lnz")
                    nc.scalar.activation(lnz[:], o_ps[64:128, :], Log)
                    rcp = rcpp.tile([64, 2 * QW], F32, tag="rcp")
                    nc.scalar.activation(rcp[:], lnz[:], Exp, scale=-1.0)
                    o_sb = osbp.tile([64, 2 * QW], F32)
                    nc.vector.tensor_mul(o_sb[:], o_ps[0:64, :], rcp[:])
                    for h, half in ((hA, slice(0, QW)), (hB, slice(QW, 2 * QW))):
                        nc.sync.dma_start(
                            outT[h * DK:(h + 1) * DK, qt * QW:(qt + 1) * QW],
                            o_sb[:, half])
    nc.compile()
    return nc


_PROG = None


def _get_prog():
    global _PROG
    if _PROG is None:
        _PROG = build_program()
    return _PROG


def _prep_in_maps(query, key, value, mask):
    query = np.asarray(query, dtype=np.float32)
    key = np.asarray(key, dtype=np.float32)
    value = np.asarray(value, dtype=np.float32)
    mask = np.asarray(mask)
    B = query.shape[0]
    bf16 = ml_dtypes.bfloat16
    hd = HPC * DK

    # mask transpose once per batch (shared by the two cores of that batch)
    mTs = [np.ascontiguousarray(mask[b, 0].T).astype(bf16) for b in range(B)]

    in_maps = []
    for c in range(N_CORES):
        b, hg = divmod(c, 2)
        cols = slice(hg * hd, (hg + 1) * hd)
        in_maps.append({
            "qT": np.ascontiguousarray(query[b][:, cols].T).astype(bf16),
            "kT": np.ascontiguousarray(key[b][:, cols].T).astype(bf16),
            "v": value[b][:, cols].astype(bf16),
            "mT": mTs[b],
        })
    return in_maps


def _unshard(results, B, s, D):
    hd = HPC * DK
    out = np.empty((B, s, D), np.float32)
    for c in range(N_CORES):
        b, hg = divmod(c, 2)
        out[b][:, hg * hd:(hg + 1) * hd] = results[c]["outT"].T
    return out


def kernel(query, key, value, mask):
    global LAST_RESULTS
    B, s, D = np.asarray(query).shape
    in_maps = _prep_in_maps(query, key, value, mask)
    nc = _get_prog()
    res = bass_utils.run_bass_kernel_spmd(
        nc, in_maps, core_ids=list(range(N_CORES)), trace=False)
    LAST_RESULTS = res
    return _unshard(res.results, B, s, D)


def benchmark(query, key, value, mask, iters=20):
    """Run the kernel on 8 cores; return (out, per_call_seconds).

    Times steady-state repeated PJRT executions with inputs pre-placed on
    device, so the measurement is NEFF execution + runtime launch overhead
    (no NTFF profiling is available under this bare axon plugin).
    """
    import time as _time
    import jax
    from jax.sharding import Mesh, PartitionSpec, NamedSharding
    from jax.experimental.shard_map import shard_map
    from concourse import bass2jax, mybir as _mybir

    B, s, D = np.asarray(query).shape
    in_maps = _prep_in_maps(query, key, value, mask)
    nc = _get_prog()
    bass2jax.install_neuronx_cc_hook()

    partition_name = (nc.partition_id_tensor.name
                      if nc.partition_id_tensor else None)
    in_names, out_names, out_avals, zero_outs = [], [], [], []
    for alloc in nc.m.functions[0].allocations:
        if not isinstance(alloc, _mybir.MemoryLocationSet):
            continue
        name = alloc.memorylocations[0].name
        if alloc.kind == "ExternalInput":
            if name != partition_name:
                in_names.append(name)
        elif alloc.kind == "ExternalOutput":
            out_names.append(name)
            shape = tuple(alloc.tensor_shape)
            dtype = _mybir.dt.np(alloc.dtype)
            out_avals.append(jax.core.ShapedArray(shape, dtype))
            zero_outs.append(np.zeros(shape, dtype))
    n_params = len(in_names)
    bind_names = list(in_names) + list(out_names)
    if partition_name is not None:
        bind_names.append(partition_name)

    def _body(*args):
        operands = list(args)
        if partition_name is not None:
            operands.append(bass2jax.partition_id_tensor())
        outs = bass2jax._bass_exec_p.bind(
            *operands, out_avals=tuple(out_avals), in_names=tuple(bind_names),
            out_names=tuple(out_names), lowering_input_output_aliases=(),
            sim_require_finite=True, sim_require_nnan=True, nc=nc)
        return tuple(outs)

    devices = jax.devices()[:N_CORES]
    mesh = Mesh(np.asarray(devices), ("core",))
    pspec = PartitionSpec("core")
    donate = tuple(range(n_params, n_params + len(out_names)))
    sharded = jax.jit(
        shard_map(_body, mesh=mesh,
                  in_specs=(pspec,) * (n_params + len(out_names)),
                  out_specs=(pspec,) * len(out_names), check_rep=False),
        donate_argnums=donate, keep_unused=True)

    sh = NamedSharding(mesh, pspec)
    dev_in = [jax.device_put(
        np.concatenate([in_maps[c][nm] for c in range(N_CORES)], axis=0), sh)
        for nm in in_names]
    dev_zero = [jax.device_put(
        np.zeros((N_CORES * z.shape[0], *z.shape[1:]), z.dtype), sh)
        for z in zero_outs]

    # Donation chain: this kernel writes every output element, so the
    # previous call's outputs are valid donated "zero" buffers.
    out_arrs = sharded(*dev_in, *dev_zero)
    jax.block_until_ready(out_arrs)
    keep = [np.asarray(a) for a in out_arrs]  # correctness copy (1st call)
    for _ in range(2):
        out_arrs = sharded(*dev_in, *out_arrs)
        jax.block_until_ready(out_arrs)
    t0 = _time.perf_counter()
    for _ in range(iters):
        out_arrs = sharded(*dev_in, *out_arrs)
    jax.block_until_ready(out_arrs)
    dt = (_time.perf_counter() - t0) / iters
    out_arrs = keep

    results = [
        {name: np.asarray(out_arrs[i]).reshape(N_CORES, *out_avals[i].shape)[c]
         for i, name in enumerate(out_names)}
        for c in range(N_CORES)]
    return _unshard(results, B, s, D), dt
